# revision 49
# baseline (speedup 1.0000x reference)
"""CosineEmbeddingLoss (B=8192, D=128) on 8 TRN2 NeuronCores.

Data-parallel over anchor rows: each core gets its [1024,128] anchor
slab plus the full [8192,128] positive matrix (bf16). The host
marshals layouts only (casts / transposes / reorders — no arithmetic):
  - "p2"  [128,8192]: chunk-local layout; for chunk k partition p holds
    positive rows [1024k+8p, 1024k+8p+8) so every DMA line is 2KB
    contiguous
  - "an"/"pdn" [128,1024]: natural tile layout (partition = row%128) of
    the anchor / matching-positive slab, for norms + diagonal
  - "at" [128,1024]: anchor slab transposed (matmul lhsT)
Per core:
  - positive norms per chunk: square (GpSimd; VectorE for the first two
    bootstrap chunks), grouped 3D reduce (VectorE), sqrt (ScalarE),
    reciprocal (VectorE), then per-row scale via broadcast-AP
    tensor_tensor (GpSimd / VectorE for bootstrap)
  - scaled rows round-trip through DRAM scratch and come back as
    contiguous p^T chunk tiles [128,1024] via xbar transpose-DMA
  - main loop: 128 bf16 matmuls [K=128, M=128, N=512] -> PSUM [128,2048]
    groups, consumed in ONE pass each by ScalarE (activation Relu +
    accum_out) or VectorE (tensor_scalar max/add + accum_out),
    interleaved in emission order to keep both engines busy
  - diagonal correction from row-dots of matching anchor/positive rows
  - final 128-partition reduction via a tiny fp32 ones-matmul
Host sums the 8 partial scalars, adds B (the +1 per diagonal term) and
divides by B*B.
"""

import contextlib
import os

import numpy as np
import ml_dtypes

import concourse.bass as bass
import concourse.tile as tile
from concourse import bacc, mybir
from concourse.bass_utils import run_bass_kernel_spmd

B, D, NCORES = 8192, 128, 8
SLAB = B // NCORES          # 1024 anchor rows per core
AT = SLAB // 128            # 8 anchor tiles
NCHUNK = 8                  # positive chunks
CH = B // NCHUNK            # 1024 positive rows per chunk
RL = CH // 128              # row-group size within a chunk partition
NBOOT = 2                   # chunks prepped on VectorE for fast start
NJG = 4                     # j-groups of 2048 columns
GRPN = 2048
MMN = 512                   # matmul free dim
NGRP = NJG * AT             # 32 psum groups
F32 = mybir.dt.float32
BF16 = mybir.dt.bfloat16

S_GROUPS = int(os.environ.get("K_S_GROUPS", "18"))
WARM_MMS = int(os.environ.get("K_WARM_MMS", "44"))
GP_SCALE = os.environ.get("K_GP_SCALE", "1") == "1"
GP_SQ = os.environ.get("K_GP_SQ", "1") == "1"

_CACHE: dict = {}


def _use_scalar(e: int) -> bool:
    # assign in PAIRS of consecutive emitted groups: back-to-back
    # ScalarE activations overlap the accumulator-read/drain, so paired
    # S-groups run ~20% faster than isolated ones
    p = e // 2
    npair = NGRP // 2
    spair = S_GROUPS // 2
    return (p * spair) % npair < spair


def _body(tc, p2_in, an_in, pdn_in, at_in, pt_in, out):
    nc = tc.nc
    Relu = mybir.ActivationFunctionType.Relu
    Sqrt = mybir.ActivationFunctionType.Sqrt
    mult, add, amax = mybir.AluOpType.mult, mybir.AluOpType.add, mybir.AluOpType.max
    sub = mybir.AluOpType.subtract
    X = mybir.AxisListType.X

    sq_eng = nc.gpsimd if GP_SQ else nc.vector
    sc_eng = nc.gpsimd if GP_SCALE else nc.vector

    ctx = contextlib.ExitStack()
    with ctx:
        singles = ctx.enter_context(tc.tile_pool(name="singles", bufs=1))
        chunks = ctx.enter_context(tc.tile_pool(name="chunks", bufs=3))
        junkp = ctx.enter_context(tc.tile_pool(name="junkp", bufs=3))
        dramp = ctx.enter_context(tc.tile_pool(name="dramp", bufs=1, space="DRAM"))
        main_ctx = contextlib.ExitStack()

        # persistent tiles (p^T as separate per-chunk tiles: transpose-DMA
        # needs a contiguous SBUF destination)
        pTc = [singles.tile([128, CH], BF16, name=f"pTc{k}")
               for k in range(NCHUNK)]
        pn_tiles = [singles.tile([128, 2 * CH], BF16, name=f"pn{k}")
                    for k in range(NCHUNK // 2)]
        aT = singles.tile([128, SLAB], BF16)
        a_nat = singles.tile([128, SLAB], BF16)
        pd_nat = singles.tile([128, SLAB], BF16)
        rsq_p = singles.tile([128, NCHUNK * RL], F32)
        ssq_p = singles.tile([128, NCHUNK * RL], F32)
        sumsq_a = singles.tile([128, AT], F32)
        rsq_a = singles.tile([128, AT], F32)
        sumsq_pd = singles.tile([128, AT], F32)
        rsq_pd = singles.tile([128, AT], F32)
        draw = singles.tile([128, AT], F32)
        dcorr = singles.tile([128, AT], F32)
        racc_s = singles.tile([128, NGRP], F32)
        racc_v = singles.tile([128, NGRP], F32)
        ones1 = singles.tile([128, 1], F32)
        tot = singles.tile([128, 1], F32)
        phat_dram = dramp.tile([B, D], BF16)

        nc.vector.memset(racc_s[:], 0.0)
        nc.vector.memset(racc_v[:], 0.0)
        nc.vector.memset(ones1[:], 1.0)
        # trigger GpSimd's one-time IRAM load early, off the critical path
        gp_warm = singles.tile([128, 8], BF16)
        nc.gpsimd.memset(gp_warm[:], 0.0)
        nc.gpsimd.tensor_tensor(
            out=gp_warm[:], in0=gp_warm[:], in1=gp_warm[:], op=mult)

        # chunk-local store view: chunk k, partition p, row group r ->
        # positive row 1024k + 8p + r (doubles: s = sub-chunk)
        pd_w2 = phat_dram[:].rearrange(
            "(dd s p r) d -> dd p s r d", p=128, r=RL, s=2)

        # raw p^T columns for the bootstrap chunks (host-marshalled)
        ptb = singles.tile([128, NBOOT * CH], BF16)
        rsq_bf = singles.tile([128, NBOOT * RL], BF16)
        rsqT = singles.tile([1, NBOOT * CH], BF16)
        onesk = singles.tile([1, 128], BF16)
        nc.vector.memset(onesk[:], 1.0)

        # ---- input DMAs (all per-partition contiguous; boot first) ----
        nc.sync.dma_start(
            out=pn_tiles[0][:], in_=p2_in[:, 0 : 2 * CH])
        nc.sync.dma_start(out=ptb[:], in_=pt_in)
        nc.sync.dma_start(out=aT[:], in_=at_in)
        nc.sync.dma_start(out=pd_nat[:], in_=pdn_in)
        nc.sync.dma_start(out=a_nat[:], in_=an_in)
        for dd in range(1, NCHUNK // 2):
            nc.sync.dma_start(
                out=pn_tiles[dd][:],
                in_=p2_in[:, dd * 2 * CH : (dd + 1) * 2 * CH])

        RL2 = 2 * RL  # row groups per double-chunk partition

        def emit_norms(dd):
            # double 0 squared on VectorE for a fast pipeline start;
            # later doubles on GpSimd (warmed by then)
            sl = slice(dd * RL2, (dd + 1) * RL2)
            sq = chunks.tile([128, 2 * CH], BF16, tag="sq")
            eng = nc.vector if dd == 0 else sq_eng
            eng.tensor_tensor(
                out=sq[:], in0=pn_tiles[dd][:], in1=pn_tiles[dd][:], op=mult)
            nc.vector.tensor_reduce(
                out=ssq_p[:, sl], in_=sq.rearrange("p (r d) -> p r d", d=D),
                axis=X, op=add)
            nc.scalar.activation(out=rsq_p[:, sl], in_=ssq_p[:, sl], func=Sqrt)
            nc.vector.reciprocal(out=rsq_p[:, sl], in_=rsq_p[:, sl])

        def emit_chunk_scale(dd):
            # scale a double-chunk on GpSimd, round-trip through DRAM,
            # transpose back as two contiguous pTc tiles
            ph = chunks.tile([128, 2 * CH], BF16, tag="ph")
            scal = rsq_p[:, dd * RL2 : (dd + 1) * RL2]
            sc_eng.tensor_tensor(
                out=ph.rearrange("p (r d) -> p r d", d=D),
                in0=pn_tiles[dd].rearrange("p (r d) -> p r d", d=D),
                in1=scal[:, :, None].broadcast_to([128, RL2, D]),
                op=mult)
            nc.sync.dma_start(
                out=pd_w2[dd],
                in_=ph.rearrange("p (s r d) -> p s r d", d=D, r=RL))
            for k in (2 * dd, 2 * dd + 1):
                nc.sync.dma_start_transpose(
                    out=pTc[k][:], in_=phat_dram[k * CH : (k + 1) * CH, :])

        # bootstrap chunks: replicate rsq across partitions via a K=1
        # ones-matmul into PSUM, then scale raw p^T columns directly —
        # skips the DRAM round-trip on the critical path
        boot_ctx = contextlib.ExitStack()
        bpsum = boot_ctx.enter_context(
            tc.tile_pool(name="bpsum", bufs=2, space="PSUM"))
        # PE HAM warmup: keep TensorE continuously busy through the prep
        # window so the main loop runs at 2.4 GHz instead of 1.2 GHz
        if WARM_MMS:
            wsrc = singles.tile([128, MMN], BF16)
            nc.gpsimd.memset(wsrc[:], 0.0)
            wps = bpsum.tile([128, MMN], F32, tag="warm")
            for _ in range(WARM_MMS):
                nc.tensor.matmul(out=wps[:], lhsT=wsrc[:, 0:128],
                                 rhs=wsrc[:], start=True, stop=True)
        emit_norms(0)
        for k in range(NBOOT):
            sl = slice(k * RL, (k + 1) * RL)
            nc.vector.tensor_copy(out=rsq_bf[:, sl], in_=rsq_p[:, sl])
            # [128, RL] (partition-major) -> [1, CH]: j_in_chunk = RL*p + r
            nc.sync.dma_start(
                out=rsqT[0:1, k * CH : (k + 1) * CH], in_=rsq_bf[:, sl])
            rep = bpsum.tile([128, CH], F32, tag="rep")
            for u in range(CH // MMN):
                nc.tensor.matmul(
                    out=rep[:, u * MMN : (u + 1) * MMN],
                    lhsT=onesk[:],
                    rhs=rsqT[0:1, k * CH + u * MMN : k * CH + (u + 1) * MMN],
                    start=True, stop=True)
            nc.vector.tensor_tensor(
                out=pTc[k][:], in0=ptb[:, k * CH : (k + 1) * CH],
                in1=rep[:], op=mult)
        for dd in range(1, NCHUNK // 2):
            emit_norms(dd)

        # ---- main loop interleaved with remaining chunk scales ----
        boot_ctx.close()
        psum = main_ctx.enter_context(
            tc.tile_pool(name="psum", bufs=2, space="PSUM"))

        def emit_group(jg, m):
            ps = psum.tile([128, GRPN], F32, tag="ps")
            for u in range(GRPN // MMN):
                ck = jg * 2 + u // 2
                off = (u % 2) * MMN
                nc.tensor.matmul(
                    out=ps[:, u * MMN : (u + 1) * MMN],
                    lhsT=aT[:, m * 128 : (m + 1) * 128],
                    rhs=pTc[ck][:, off : off + MMN],
                    start=True, stop=True)
            idx = m * NJG + jg
            junk = junkp.tile([128, GRPN], BF16, tag="junk")
            if _use_scalar(jg * AT + m):
                nc.scalar.activation(
                    out=junk[:], in_=ps[:], func=Relu,
                    accum_out=racc_s[:, idx : idx + 1])
            else:
                nc.vector.tensor_scalar(
                    out=junk[:], in0=ps[:], scalar1=0.0, scalar2=0.0,
                    op0=amax, op1=add,
                    accum_out=racc_v[:, idx : idx + 1])

        anchor_emitted = [False]

        def emit_anchor_diag():
            sqa = chunks.tile([128, CH], BF16, tag="asq")
            sq_eng.tensor_tensor(
                out=sqa[:], in0=a_nat[:], in1=a_nat[:], op=mult)
            nc.vector.tensor_reduce(
                out=sumsq_a[:], in_=sqa.rearrange("p (t d) -> p t d", d=D),
                axis=X, op=add)
            nc.scalar.activation(out=rsq_a[:], in_=sumsq_a[:], func=Sqrt)
            nc.vector.reciprocal(out=rsq_a[:], in_=rsq_a[:])
            sqpd = chunks.tile([128, CH], BF16, tag="asq")
            sq_eng.tensor_tensor(
                out=sqpd[:], in0=pd_nat[:], in1=pd_nat[:], op=mult)
            nc.vector.tensor_reduce(
                out=sumsq_pd[:], in_=sqpd.rearrange("p (t d) -> p t d", d=D),
                axis=X, op=add)
            nc.scalar.activation(out=rsq_pd[:], in_=sumsq_pd[:], func=Sqrt)
            nc.vector.reciprocal(out=rsq_pd[:], in_=rsq_pd[:])
            dj = chunks.tile([128, CH], BF16, tag="asq")
            sq_eng.tensor_tensor(
                out=dj[:], in0=a_nat[:], in1=pd_nat[:], op=mult)
            nc.vector.tensor_reduce(
                out=draw[:], in_=dj.rearrange("p (t d) -> p t d", d=D),
                axis=X, op=add)
            # dcos = draw * rsq_a * rsq_pd ; dcorr = dcos + relu(dcos)
            nc.vector.tensor_mul(draw[:], draw[:], rsq_a[:])
            nc.vector.tensor_mul(draw[:], draw[:], rsq_pd[:])
            nc.scalar.activation(out=dcorr[:], in_=draw[:], func=Relu)
            nc.vector.tensor_add(dcorr[:], dcorr[:], draw[:])
            anchor_emitted[0] = True

        for jg in range(NJG):
            for m in range(AT):
                emit_group(jg, m)
            if 1 <= jg + 1 < NCHUNK // 2:
                emit_chunk_scale(jg + 1)
            if jg == 2 and not anchor_emitted[0]:
                emit_anchor_diag()

        # ---- combine ----
        nc.vector.tensor_add(racc_s[:], racc_s[:], racc_v[:])
        rowsum = singles.tile([128, AT], F32)
        nc.vector.tensor_reduce(
            out=rowsum[:],
            in_=racc_s.rearrange("p (m g) -> p m g", g=NJG),
            axis=X, op=add)
        nc.vector.tensor_mul(rowsum[:], rowsum[:], rsq_a[:])
        nc.vector.tensor_tensor(rowsum[:], rowsum[:], dcorr[:], op=sub)
        nc.vector.tensor_reduce(out=tot[:], in_=rowsum[:], axis=X, op=add)

        main_ctx.close()
        tail = ctx.enter_context(tc.tile_pool(name="tail", bufs=1, space="PSUM"))
        ps1 = tail.tile([1, 1], F32)
        nc.tensor.matmul(out=ps1[:], lhsT=tot[:], rhs=ones1[:],
                         start=True, stop=True)
        res = singles.tile([1, 1], F32)
        nc.vector.tensor_copy(out=res[:], in_=ps1[:])
        nc.sync.dma_start(out=out[:], in_=res[:])


def _build():
    nc = bacc.Bacc("TRN2", target_bir_lowering=False, debug=False,
                   num_devices=NCORES)
    p2_in = nc.declare_dram_parameter("p2", [128, B], BF16, isOutput=False)
    an_in = nc.declare_dram_parameter("an", [128, SLAB], BF16, isOutput=False)
    pdn_in = nc.declare_dram_parameter("pdn", [128, SLAB], BF16, isOutput=False)
    at_in = nc.declare_dram_parameter("at", [128, SLAB], BF16, isOutput=False)
    pt_in = nc.declare_dram_parameter("pt", [128, NBOOT * CH], BF16,
                                      isOutput=False)
    out = nc.declare_dram_parameter("out", [1, 1], F32, isOutput=True)
    with tile.TileContext(nc) as tc:
        _body(tc, p2_in[:], an_in[:], pdn_in[:], at_in[:], pt_in[:], out[:])
    nc.compile()
    return nc


def _marshal(p16: np.ndarray, a16: np.ndarray):
    """Host-side layout marshalling (no arithmetic)."""
    # p2: chunk k, partition p holds rows [1024k+8p, 1024k+8p+8)
    p2 = np.ascontiguousarray(
        p16.reshape(NCHUNK, 128, RL, D).transpose(1, 0, 2, 3).reshape(128, B))
    pt = np.ascontiguousarray(p16[: NBOOT * CH].T)
    ins = []
    for c in range(NCORES):
        a_sl = a16[c * SLAB : (c + 1) * SLAB]
        pd_sl = p16[c * SLAB : (c + 1) * SLAB]
        an = np.ascontiguousarray(
            a_sl.reshape(AT, 128, D).transpose(1, 0, 2).reshape(128, SLAB))
        pdn = np.ascontiguousarray(
            pd_sl.reshape(AT, 128, D).transpose(1, 0, 2).reshape(128, SLAB))
        at = np.ascontiguousarray(a_sl.T)
        ins.append({"p2": p2, "an": an, "pdn": pdn, "at": at, "pt": pt})
    return ins


def kernel(hid_positive: np.ndarray, hid_anchor: np.ndarray, **run_kwargs):
    if "nc" not in _CACHE:
        _CACHE["nc"] = _build()
    nc = _CACHE["nc"]
    p16 = np.asarray(hid_positive, dtype=np.float32).astype(ml_dtypes.bfloat16)
    a16 = np.asarray(hid_anchor, dtype=np.float32).astype(ml_dtypes.bfloat16)
    in_maps = _marshal(p16, a16)
    res = run_bass_kernel_spmd(nc, in_maps, core_ids=list(range(NCORES)),
                               **run_kwargs)
    s = sum(float(res.results[c]["out"][0, 0]) for c in range(NCORES))
    loss = np.float32((s + B) / (float(B) * float(B)))
    if run_kwargs:
        _CACHE["last_result"] = res
    return np.asarray(loss, dtype=np.float32)
